# revision 11
# baseline (speedup 1.0000x reference)
"""Multi-head attention (B=2, T=S=2048, E=1024, H=16, D=64) on 8 NeuronCores.

Sharding: core = (batch, head-group-of-4).  Each core computes the full
attention for 4 heads of one batch plus that slice's out-projection
contribution; the host sums the 4 partials per batch.

Device math (per core, all matmuls bf16, accumulation fp32 in PSUM):
  qT = (scale*Wq_c) @ x_q     -> [256, T]   (d on partitions)
  kT = Wk_c @ x_k             -> [256, S]
  v  = x_v @ Wv_c^T           -> [S, 256]   (s on partitions), plus ones col
  scoresT[s,t] = kT^T·qT      (K=64 per head)
  p = exp(scoresT) * ebm      (ebm = exp(bias^T)*keep_mask, host-precomputed;
                               softmax needs no max-subtraction: scores are O(10))
  [out_unT; sums] = [v|1]^T @ p   (ones column fuses the softmax denominator)
  onorm = out_unT * bcast(1/sums) (gpsimd partition-broadcast)
  out_partial = onorm^T @ Wo_c^T  -> [T, E] fp32
"""

import numpy as np
import ml_dtypes

B, T, S, E = 2, 2048, 2048, 1024
H, D = 16, 64
SCALING = float(D) ** -0.5
HEADS = 4              # heads per core
JD = HEADS * D         # 256 projected dims per core
NCORES = 8

BF16 = ml_dtypes.bfloat16


def _build_nc():
    import concourse.bass as bass
    import concourse.mybir as mybir
    import concourse.tile as tile
    from concourse import bacc
    from contextlib import ExitStack

    DT = mybir.dt.bfloat16
    F32 = mybir.dt.float32
    Act = mybir.ActivationFunctionType

    EC = E // 128        # 8 contraction chunks for projections
    MC = JD // 128       # 2 partition-chunks of the per-core head dims
    SC = S // 128        # 16 key chunks
    NT512 = T // 512     # 4
    NT128 = T // 128     # 16
    NE512 = E // 512     # 2

    nc = bacc.Bacc("TRN2", target_bir_lowering=False, debug=False)

    xqT = nc.dram_tensor("xqT", [E, T], DT, kind="ExternalInput")
    xkT = nc.dram_tensor("xkT", [E, S], DT, kind="ExternalInput")
    xvT = nc.dram_tensor("xvT", [E, S], DT, kind="ExternalInput")
    ebm = nc.dram_tensor("ebm", [S, T], DT, kind="ExternalInput")
    wqT = nc.dram_tensor("wqT", [E, JD], DT, kind="ExternalInput")
    wkT = nc.dram_tensor("wkT", [E, JD], DT, kind="ExternalInput")
    wvT = nc.dram_tensor("wvT", [E, JD], DT, kind="ExternalInput")
    woT = nc.dram_tensor("woT", [JD, E], DT, kind="ExternalInput")
    bqv = nc.dram_tensor("bqv", [128, MC], F32, kind="ExternalInput")
    bkv = nc.dram_tensor("bkv", [128, MC], F32, kind="ExternalInput")
    out_p = nc.dram_tensor("out_p", [T, E], F32, kind="ExternalOutput")

    with tile.TileContext(nc) as tc, ExitStack() as ctx:
        const = ctx.enter_context(tc.tile_pool(name="const", bufs=1))
        persist = ctx.enter_context(tc.tile_pool(name="persist", bufs=1))

        # ---- constants / persistent tiles ----
        wo_sb = const.tile([128, MC, E], DT)
        nc.sync.dma_start(out=wo_sb[:], in_=woT.rearrange("(c p) e -> p c e", p=128))
        bq_sb = const.tile([128, MC], F32)
        nc.sync.dma_start(out=bq_sb[:], in_=bqv[:, :])
        bk_sb = const.tile([128, MC], F32)
        nc.sync.dma_start(out=bk_sb[:], in_=bkv[:, :])

        qT_sb = persist.tile([128, MC, T], DT)
        kT_sb = persist.tile([128, MC, S], DT)
        vone_sb = persist.tile([128, SC, HEADS, D + 1], DT)
        onorm_sb = persist.tile([128, MC, T], DT)
        ebm_sb = persist.tile([128, SC, T], DT)

        nc.vector.memset(vone_sb[:, :, :, D : D + 1], 1.0)

        # ---- phase 1: input projections ----
        with (
            nc.named_scope("proj"),
            tc.tile_pool(name="wpool", bufs=1) as wpool,
            tc.tile_pool(name="xpool", bufs=2) as xpool,
            tc.tile_pool(name="psA", bufs=4, space="PSUM") as psA,
        ):
            wq_sb = wpool.tile([128, EC, JD], DT)
            nc.sync.dma_start(
                out=wq_sb[:], in_=wqT.rearrange("(c p) j -> p c j", p=128)
            )
            wk_sb = wpool.tile([128, EC, JD], DT)
            nc.sync.dma_start(
                out=wk_sb[:], in_=wkT.rearrange("(c p) j -> p c j", p=128)
            )
            wv_sb = wpool.tile([128, EC, JD], DT)
            nc.sync.dma_start(
                out=wv_sb[:], in_=wvT.rearrange("(c p) j -> p c j", p=128)
            )
            for w_sb, b_sb, xdram, dst in (
                (wq_sb, bq_sb, xqT, qT_sb),
                (wk_sb, bk_sb, xkT, kT_sb),
            ):
                xf = xpool.tile([128, EC, T], DT, tag="xfull", name=f"xf_{dst.name}")
                nc.sync.dma_start(
                    out=xf[:], in_=xdram.rearrange("(c p) t -> p c t", p=128)
                )
                for mc in range(MC):
                    for tq in range(NT512):
                        ps = psA.tile([128, 512], F32, tag="pqk")
                        for ec in range(EC):
                            nc.tensor.matmul(
                                ps[:],
                                lhsT=w_sb[:, ec, mc * 128 : (mc + 1) * 128],
                                rhs=xf[:, ec, tq * 512 : (tq + 1) * 512],
                                start=(ec == 0),
                                stop=(ec == EC - 1),
                            )
                        nc.scalar.activation(
                            dst[:, mc, tq * 512 : (tq + 1) * 512],
                            ps[:],
                            Act.Identity,
                            bias=b_sb[:, mc : mc + 1],
                        )

            xf = xpool.tile([128, EC, S], DT, tag="xfull")
            nc.sync.dma_start(out=xf[:], in_=xvT.rearrange("(c p) t -> p c t", p=128))
            for sc in range(SC):
                ps = psA.tile([128, JD], F32, tag="pv")
                for ec in range(EC):
                    nc.tensor.matmul(
                        ps[:],
                        lhsT=xf[:, ec, sc * 128 : (sc + 1) * 128],
                        rhs=wv_sb[:, ec, :],
                        start=(ec == 0),
                        stop=(ec == EC - 1),
                    )
                nc.vector.tensor_copy(
                    vone_sb[:, sc, :, 0:D],
                    ps.rearrange("p (h d) -> p h d", h=HEADS),
                )

        # ebm load (phase 2 input); chunked so early chunks unblock head 0
        for sc in range(SC):
            nc.sync.dma_start(
                out=ebm_sb[:, sc, :],
                in_=ebm[sc * 128 : (sc + 1) * 128, :],
            )

        # ---- phase 2: attention per head ----
        with (
            nc.named_scope("attn"),
            tc.tile_pool(name="flow", bufs=4) as flow,
            tc.tile_pool(name="dpool", bufs=2) as dpool,
            tc.tile_pool(name="psB", bufs=2, space="PSUM") as psB,
            tc.tile_pool(name="psPV", bufs=4, space="PSUM") as psPV,
        ):
            for h in range(HEADS):
                mc, par = h // 2, (h % 2) * 64
                kTh = kT_sb[par : par + 64, mc, :]
                qTh = qT_sb[par : par + 64, mc, :]
                pvp = [
                    psPV.tile([D + 1, 512], F32, tag="pvh", name=f"pvp{h}_{i}")
                    for i in range(NT512)
                ]
                for sc in range(SC):
                    for th in range(T // 1024):
                        ps = psB.tile([128, 1024], F32, tag="sc")
                        for j in range(2):
                            t0 = th * 1024 + j * 512
                            nc.tensor.matmul(
                                ps[:, j * 512 : (j + 1) * 512],
                                lhsT=kTh[:, sc * 128 : (sc + 1) * 128],
                                rhs=qTh[:, t0 : t0 + 512],
                                start=True,
                                stop=True,
                            )
                        pe = flow.tile([128, 1024], DT, tag="pe")
                        nc.scalar.activation(pe[:], ps[:], Act.Exp)
                        pw = flow.tile([128, 1024], DT, tag="pw")
                        nc.vector.tensor_mul(
                            pw[:],
                            pe[:],
                            ebm_sb[:, sc, th * 1024 : (th + 1) * 1024],
                        )
                        for j in range(2):
                            nc.tensor.matmul(
                                pvp[th * 2 + j][:],
                                lhsT=vone_sb[:, sc, h, :],
                                rhs=pw[:, j * 512 : (j + 1) * 512],
                                start=(sc == 0),
                                stop=(sc == SC - 1),
                            )
                for tq in range(NT512):
                    oun = dpool.tile([D + 1, 512], F32, tag="oun")
                    nc.vector.tensor_copy(oun[:], pvp[tq][:])
                    rec = dpool.tile([1, 512], F32, tag="rec")
                    nc.vector.reciprocal(rec[:], oun[D : D + 1, :])
                    bc = dpool.tile([64, 512], F32, tag="bc")
                    nc.gpsimd.partition_broadcast(bc[:], rec[:], channels=64)
                    nc.vector.tensor_mul(
                        onorm_sb[par : par + 64, mc, tq * 512 : (tq + 1) * 512],
                        oun[0:D, :],
                        bc[:],
                    )

        # ---- phase 3: out-projection ----
        with (
            nc.named_scope("oproj"),
            tc.tile_pool(name="outp", bufs=4) as outp,
            tc.tile_pool(name="psC", bufs=4, space="PSUM") as psC,
        ):
            for ti in range(NT128):
                for ei in range(NE512):
                    po = psC.tile([128, 512], F32, tag="po")
                    for mc in range(MC):
                        nc.tensor.matmul(
                            po[:],
                            lhsT=onorm_sb[:, mc, ti * 128 : (ti + 1) * 128],
                            rhs=wo_sb[:, mc, ei * 512 : (ei + 1) * 512],
                            start=(mc == 0),
                            stop=(mc == MC - 1),
                        )
                    ot = outp.tile([128, 512], F32, tag="ot")
                    nc.vector.tensor_copy(ot[:], po[:])
                    nc.sync.dma_start(
                        out=out_p[ti * 128 : (ti + 1) * 128, ei * 512 : (ei + 1) * 512],
                        in_=ot[:],
                    )

    nc.compile()
    return nc


_NC = None
_LAST_RES = None


def _get_nc():
    global _NC
    if _NC is None:
        _NC = _build_nc()
    return _NC


def kernel(query, key, value, attn_bias, key_padding_mask,
           in_proj_w, in_proj_b, out_proj_w, out_proj_b):
    from concourse.bass_utils import run_bass_kernel_spmd

    query = np.asarray(query, np.float32)
    key = np.asarray(key, np.float32)
    value = np.asarray(value, np.float32)
    attn_bias = np.asarray(attn_bias, np.float32)
    key_padding_mask = np.asarray(key_padding_mask, bool)
    in_proj_w = np.asarray(in_proj_w, np.float32)
    in_proj_b = np.asarray(in_proj_b, np.float32)
    out_proj_w = np.asarray(out_proj_w, np.float32)
    out_proj_b = np.asarray(out_proj_b, np.float32)

    w_q, w_k, w_v = in_proj_w[:E], in_proj_w[E : 2 * E], in_proj_w[2 * E :]
    b_q, b_k, b_v = in_proj_b[:E], in_proj_b[E : 2 * E], in_proj_b[2 * E :]

    ebm_base = np.exp(attn_bias[0]).T  # [S, T]
    keep = (~key_padding_mask).astype(np.float32)  # [B, S]

    xT = {}
    for b in range(B):
        xT[b] = (
            query[b].T.astype(BF16),
            key[b].T.astype(BF16),
            value[b].T.astype(BF16),
            (ebm_base * keep[b][:, None]).astype(BF16),
        )

    in_maps = []
    for core in range(NCORES):
        b, hg = core // 4, core % 4
        rows = slice(hg * JD, (hg + 1) * JD)
        qT, kT, vT, ebm_b = xT[b]
        in_maps.append({
            "xqT": qT, "xkT": kT, "xvT": vT, "ebm": ebm_b,
            "wqT": np.ascontiguousarray((SCALING * w_q[rows]).T).astype(BF16),
            "wkT": np.ascontiguousarray(w_k[rows].T).astype(BF16),
            "wvT": np.ascontiguousarray(w_v[rows].T).astype(BF16),
            "woT": np.ascontiguousarray(out_proj_w[:, rows].T).astype(BF16),
            "bqv": np.ascontiguousarray(
                (SCALING * b_q[rows]).reshape(JD // 128, 128).T),
            "bkv": np.ascontiguousarray(b_k[rows].reshape(JD // 128, 128).T),
        })

    nc = _get_nc()
    import os
    trace = os.environ.get("KERNEL_TRACE", "") == "1"
    kwargs = {}
    if trace:
        kwargs["tmpdir"] = os.environ.get("KERNEL_TRACE_DIR") or None
    res = run_bass_kernel_spmd(
        nc, in_maps, core_ids=list(range(NCORES)), trace=trace, **kwargs
    )
    global _LAST_RES
    _LAST_RES = res

    out = np.zeros((B, T, E), np.float32)
    for core in range(NCORES):
        out[core // 4] += res.results[core]["out_p"]
    out += (out_proj_b + out_proj_w @ b_v)[None, None, :]
    return out


# revision 24
# speedup vs baseline: 38144.6315x; 38144.6315x over previous
"""Multi-head attention (B=2, T=S=2048, E=1024, H=16, D=64) on 8 NeuronCores.

Sharding: core = (batch, head-group-of-4).  Each core computes the full
attention for 4 heads of one batch plus that slice's out-projection
contribution; the host sums the 4 partials per batch.

Device math (per core, all matmuls bf16, accumulation fp32 in PSUM):
  qT = (scale*Wq_c) @ x_q     -> [256, T]   (d on partitions)
  kT = Wk_c @ x_k             -> [256, S]
  v  = x_v @ Wv_c^T           -> [S, 256]   (s on partitions), plus ones col
  scoresT[s,t] = kT^T·qT      (K=64 per head)
  p = exp(scoresT) * ebm      (ebm = exp(bias^T)*keep_mask, host-precomputed;
                               softmax needs no max-subtraction: scores are O(10))
  [out_unT; sums] = [v|1]^T @ p   (ones column fuses the softmax denominator)
  onorm = out_unT * bcast(1/sums) (gpsimd partition-broadcast)
  out_partial = onorm^T @ Wo_c^T  -> [T, E] fp32
"""

import numpy as np
import ml_dtypes

B, T, S, E = 2, 2048, 2048, 1024
H, D = 16, 64
SCALING = float(D) ** -0.5
HEADS = 4              # heads per core
JD = HEADS * D         # 256 projected dims per core
NCORES = 8

BF16 = ml_dtypes.bfloat16


def _build_nc():
    import concourse.bass as bass
    import concourse.mybir as mybir
    import concourse.tile as tile
    from concourse import bacc
    from contextlib import ExitStack

    DT = mybir.dt.bfloat16
    F32 = mybir.dt.float32
    Act = mybir.ActivationFunctionType

    EC = E // 128        # 8 contraction chunks for projections
    MC = JD // 128       # 2 partition-chunks of the per-core head dims
    SC = S // 128        # 16 key chunks
    NT512 = T // 512     # 4
    NT128 = T // 128     # 16
    NE512 = E // 512     # 2

    nc = bacc.Bacc("TRN2", target_bir_lowering=False, debug=False)

    xqT = nc.dram_tensor("xqT", [E, T], DT, kind="ExternalInput")
    xkT = nc.dram_tensor("xkT", [E, S], DT, kind="ExternalInput")
    xvT = nc.dram_tensor("xvT", [E, S], DT, kind="ExternalInput")
    ebm = nc.dram_tensor("ebm", [S, T], DT, kind="ExternalInput")
    wqT = nc.dram_tensor("wqT", [E, JD], DT, kind="ExternalInput")
    wkT = nc.dram_tensor("wkT", [E, JD], DT, kind="ExternalInput")
    wvT = nc.dram_tensor("wvT", [E, JD], DT, kind="ExternalInput")
    woT = nc.dram_tensor("woT", [JD, E], DT, kind="ExternalInput")
    bqv = nc.dram_tensor("bqv", [128, MC], F32, kind="ExternalInput")
    bkv = nc.dram_tensor("bkv", [128, MC], F32, kind="ExternalInput")
    out_p = nc.dram_tensor("out_p", [T, E], F32, kind="ExternalOutput")

    with tile.TileContext(nc) as tc, ExitStack() as ctx:
        const = ctx.enter_context(tc.tile_pool(name="const", bufs=1))
        persist = ctx.enter_context(tc.tile_pool(name="persist", bufs=1))

        # ---- constants / persistent tiles ----
        wo_sb = const.tile([128, MC, E], DT)
        bq_sb = const.tile([128, MC], F32)
        nc.sync.dma_start(out=bq_sb[:], in_=bqv[:, :])
        bk_sb = const.tile([128, MC], F32)
        nc.sync.dma_start(out=bk_sb[:], in_=bkv[:, :])

        qT_sb = persist.tile([128, MC, T], DT)
        kT_sb = persist.tile([128, MC, S], DT)
        vone_sb = persist.tile([128, SC, HEADS, D + 1], DT)
        onorm_sb = persist.tile([128, MC, T], DT)
        ebm_sb = persist.tile([128, SC, T], DT)

        nc.vector.memset(vone_sb[:, :, :, D : D + 1], 1.0)

        # ---- phase 1: input projections ----
        with (
            nc.named_scope("proj"),
            tc.tile_pool(name="wpool", bufs=1) as wpool,
            tc.tile_pool(name="xpool", bufs=4) as xpool,
            tc.tile_pool(name="psA", bufs=4, space="PSUM") as psA,
        ):
            wq_sb = wpool.tile([128, EC, JD], DT)
            nc.sync.dma_start(
                out=wq_sb[:], in_=wqT.rearrange("(c p) j -> p c j", p=128)
            )
            wk_sb = wpool.tile([128, EC, JD], DT)
            nc.sync.dma_start(
                out=wk_sb[:], in_=wkT.rearrange("(c p) j -> p c j", p=128)
            )
            wv_sb = wpool.tile([128, EC, JD], DT)
            nc.sync.dma_start(
                out=wv_sb[:], in_=wvT.rearrange("(c p) j -> p c j", p=128)
            )
            HEC = EC // 2
            for w_sb, b_sb, xdram, dst in (
                (wq_sb, bq_sb, xqT, qT_sb),
                (wk_sb, bk_sb, xkT, kT_sb),
            ):
                xr = xdram.rearrange("(c p) t -> c p t", p=128)
                xh = []
                for half in range(2):
                    t = xpool.tile(
                        [128, HEC, T], DT, tag="xhalf", name=f"xf_{dst.name}{half}"
                    )
                    for q4 in range(2):
                        nc.sync.dma_start(
                            out=t[:, q4 * (HEC // 2) : (q4 + 1) * (HEC // 2), :],
                            in_=xr[
                                half * HEC + q4 * (HEC // 2) : half * HEC
                                + (q4 + 1) * (HEC // 2)
                            ].rearrange("c p t -> p c t"),
                        )
                    xh.append(t)
                for mc in range(MC):
                    for tq in range(NT512):
                        ps = psA.tile([128, 512], F32, tag="pqk")
                        for ec in range(EC):
                            nc.tensor.matmul(
                                ps[:],
                                lhsT=w_sb[:, ec, mc * 128 : (mc + 1) * 128],
                                rhs=xh[ec // HEC][
                                    :, ec % HEC, tq * 512 : (tq + 1) * 512
                                ],
                                start=(ec == 0),
                                stop=(ec == EC - 1),
                            )
                        nc.scalar.activation(
                            dst[:, mc, tq * 512 : (tq + 1) * 512],
                            ps[:],
                            Act.Identity,
                            bias=b_sb[:, mc : mc + 1],
                        )

            xr = xvT.rearrange("(c p) t -> c p t", p=128)
            xh = []
            for half in range(2):
                t = xpool.tile([128, HEC, S], DT, tag="xhalf", name=f"xf_v{half}")
                nc.sync.dma_start(
                    out=t[:],
                    in_=xr[half * HEC : (half + 1) * HEC].rearrange("c p t -> p c t"),
                )
                xh.append(t)
            for sc in range(SC):
                ps = psA.tile([128, JD], F32, tag="pv")
                for ec in range(EC):
                    nc.tensor.matmul(
                        ps[:],
                        lhsT=xh[ec // HEC][:, ec % HEC, sc * 128 : (sc + 1) * 128],
                        rhs=wv_sb[:, ec, :],
                        start=(ec == 0),
                        stop=(ec == EC - 1),
                    )
                nc.vector.tensor_copy(
                    vone_sb[:, sc, :, 0:D],
                    ps.rearrange("p (h d) -> p h d", h=HEADS),
                )

        # ebm load (phase 2 input); chunked so early chunks unblock head 0
        for sc in range(SC):
            nc.sync.dma_start(
                out=ebm_sb[:, sc, :],
                in_=ebm[sc * 128 : (sc + 1) * 128, :],
            )

        nc.sync.dma_start(out=wo_sb[:], in_=woT.rearrange("(c p) e -> p c e", p=128))

        # ---- phase 2+3: attention (head pairs) + fused out-projection ----
        with (
            nc.named_scope("attn"),
            tc.tile_pool(name="flow", bufs=8) as flow,
            tc.tile_pool(name="dpool", bufs=6) as dpool,
            tc.tile_pool(name="outp", bufs=4) as outp,
            tc.tile_pool(name="psB", bufs=2, space="PSUM") as psB,
            tc.tile_pool(name="psPV", bufs=4, space="PSUM") as psPV,
        ):
            # heads processed in pairs (2hp, 2hp+1): the two K=64 score
            # matmuls land on disjoint PE row-groups (partitions 0-63 /
            # 64-127) and run concurrently.  t is processed in halves of
            # 1024 so scores (4 banks) + the pair's PV accumulators
            # (4 banks) fit PSUM exactly.  After both pairs finish a
            # t-half, its out-projection runs (PSUM slots shared with the
            # scores tag), overlapping the next t-half's attention.
            for th in range(T // 1024):
                for hp in range(HEADS // 2):
                    mc = hp
                    pvp = {
                        (hh, j): psPV.tile(
                            [D + 1, 512], F32, tag="pvh", name=f"pvp{hp}_{th}_{hh}{j}"
                        )
                        for hh in range(2)
                        for j in range(2)
                    }
                    for sc in range(SC):
                        for j in range(2):
                            t0 = th * 1024 + j * 512
                            ps = psB.tile([128, 1024], F32, tag="sc")
                            for hh in range(2):
                                par = hh * 64
                                nc.tensor.matmul(
                                    ps[:, hh * 512 : (hh + 1) * 512],
                                    lhsT=kT_sb[
                                        par : par + 64, mc, sc * 128 : (sc + 1) * 128
                                    ],
                                    rhs=qT_sb[par : par + 64, mc, t0 : t0 + 512],
                                    start=True,
                                    stop=True,
                                )
                            pe = flow.tile([128, 1024], DT, tag="pe")
                            nc.scalar.activation(pe[:], ps[:], Act.Exp)
                            pw = flow.tile([128, 1024], DT, tag="pw")
                            nc.vector.tensor_mul(
                                pw.rearrange("p (c t) -> p c t", c=2),
                                pe.rearrange("p (c t) -> p c t", c=2),
                                ebm_sb[:, sc, t0 : t0 + 512].rearrange(
                                    "p (c t) -> p c t", c=1
                                ).broadcast_to([128, 2, 512]),
                            )
                            for hh in range(2):
                                nc.tensor.matmul(
                                    pvp[(hh, j)][:],
                                    lhsT=vone_sb[:, sc, 2 * hp + hh, :],
                                    rhs=pw[:, hh * 512 : (hh + 1) * 512],
                                    start=(sc == 0),
                                    stop=(sc == SC - 1),
                                )
                    for hh in range(2):
                        par = hh * 64
                        for j in range(2):
                            tq = th * 2 + j
                            pv = pvp[(hh, j)]
                            rec = dpool.tile([1, 512], F32, tag="rec")
                            nc.vector.reciprocal(rec[:], pv[D : D + 1, :])
                            bc = dpool.tile([64, 512], F32, tag="bc")
                            nc.gpsimd.partition_broadcast(bc[:], rec[:], channels=64)
                            nc.vector.tensor_mul(
                                onorm_sb[
                                    par : par + 64, mc, tq * 512 : (tq + 1) * 512
                                ],
                                pv[0:D, :],
                                bc[:],
                            )

                # out-projection for this t-half (all 4 heads drained);
                # uses the PV psum slots, which are free here, so it
                # overlaps the next t-half's scores/exp instead of
                # contending for their psum.
                for ti in range(th * 8, (th + 1) * 8):
                    ot = outp.tile([128, 1024], F32, tag="ot")
                    for ei in range(NE512):
                        po = psPV.tile(
                            [128, 512], F32, tag="pvh", name=f"po_{ti}_{ei}"
                        )
                        for mc2 in range(MC):
                            nc.tensor.matmul(
                                po[:],
                                lhsT=onorm_sb[:, mc2, ti * 128 : (ti + 1) * 128],
                                rhs=wo_sb[:, mc2, ei * 512 : (ei + 1) * 512],
                                start=(mc2 == 0),
                                stop=(mc2 == MC - 1),
                            )
                        nc.vector.tensor_copy(
                            ot[:, ei * 512 : (ei + 1) * 512], po[:]
                        )
                    nc.sync.dma_start(
                        out=out_p[ti * 128 : (ti + 1) * 128, :],
                        in_=ot[:],
                    )

    nc.compile()
    return nc


_NC = None
_LAST_RES = None


def _get_nc():
    global _NC
    if _NC is None:
        _NC = _build_nc()
    return _NC


def kernel(query, key, value, attn_bias, key_padding_mask,
           in_proj_w, in_proj_b, out_proj_w, out_proj_b):
    from concourse.bass_utils import run_bass_kernel_spmd

    query = np.asarray(query, np.float32)
    key = np.asarray(key, np.float32)
    value = np.asarray(value, np.float32)
    attn_bias = np.asarray(attn_bias, np.float32)
    key_padding_mask = np.asarray(key_padding_mask, bool)
    in_proj_w = np.asarray(in_proj_w, np.float32)
    in_proj_b = np.asarray(in_proj_b, np.float32)
    out_proj_w = np.asarray(out_proj_w, np.float32)
    out_proj_b = np.asarray(out_proj_b, np.float32)

    w_q, w_k, w_v = in_proj_w[:E], in_proj_w[E : 2 * E], in_proj_w[2 * E :]
    b_q, b_k, b_v = in_proj_b[:E], in_proj_b[E : 2 * E], in_proj_b[2 * E :]

    ebm_base = np.exp(attn_bias[0]).T  # [S, T]
    keep = (~key_padding_mask).astype(np.float32)  # [B, S]

    xT = {}
    for b in range(B):
        xT[b] = (
            query[b].T.astype(BF16),
            key[b].T.astype(BF16),
            value[b].T.astype(BF16),
            (ebm_base * keep[b][:, None]).astype(BF16),
        )

    in_maps = []
    for core in range(NCORES):
        b, hg = core // 4, core % 4
        rows = slice(hg * JD, (hg + 1) * JD)
        qT, kT, vT, ebm_b = xT[b]
        in_maps.append({
            "xqT": qT, "xkT": kT, "xvT": vT, "ebm": ebm_b,
            "wqT": np.ascontiguousarray((SCALING * w_q[rows]).T).astype(BF16),
            "wkT": np.ascontiguousarray(w_k[rows].T).astype(BF16),
            "wvT": np.ascontiguousarray(w_v[rows].T).astype(BF16),
            "woT": np.ascontiguousarray(out_proj_w[:, rows].T).astype(BF16),
            "bqv": np.ascontiguousarray(
                (SCALING * b_q[rows]).reshape(JD // 128, 128).T),
            "bkv": np.ascontiguousarray(b_k[rows].reshape(JD // 128, 128).T),
        })

    nc = _get_nc()
    import os
    trace = os.environ.get("KERNEL_TRACE", "") == "1"
    kwargs = {}
    if trace:
        kwargs["tmpdir"] = os.environ.get("KERNEL_TRACE_DIR") or None
    res = run_bass_kernel_spmd(
        nc, in_maps, core_ids=list(range(NCORES)), trace=trace, **kwargs
    )
    global _LAST_RES
    _LAST_RES = res

    out = np.zeros((B, T, E), np.float32)
    for core in range(NCORES):
        out[core // 4] += res.results[core]["out_p"]
    out += (out_proj_b + out_proj_w @ b_v)[None, None, :]
    return out


# revision 31
# speedup vs baseline: 58325.5469x; 1.5291x over previous
"""Multi-head attention (B=2, T=S=2048, E=1024, H=16, D=64) on 8 NeuronCores.

Sharding: core = (batch, head-group-of-4).  Each core computes the full
attention for 4 heads of one batch plus that slice's out-projection
contribution; the host sums the 4 partials per batch.

Device math (per core, all matmuls bf16, accumulation fp32 in PSUM):
  qT = (scale*Wq_c) @ x_q     -> [256, T]   (d on partitions)
  kT = Wk_c @ x_k             -> [256, S]
  v  = x_v @ Wv_c^T           -> [S, 256]   (s on partitions), plus ones col
  scoresT[s,t] = kT^T·qT      (K=64 per head)
  p = exp(scoresT) * ebm      (ebm = exp(bias^T)*keep_mask, host-precomputed;
                               softmax needs no max-subtraction: scores are O(10))
  [out_unT; sums] = [v|1]^T @ p   (ones column fuses the softmax denominator)
  onorm = out_unT * bcast(1/sums) (gpsimd partition-broadcast)
  out_partial = onorm^T @ Wo_c^T  -> [T, E] fp32
"""

import numpy as np
import ml_dtypes

B, T, S, E = 2, 2048, 2048, 1024
H, D = 16, 64
SCALING = float(D) ** -0.5
HEADS = 4              # heads per core
JD = HEADS * D         # 256 projected dims per core
NCORES = 8

BF16 = ml_dtypes.bfloat16


def _build_nc(SP=S):
    """SP = padded count of unmasked (kept) keys, multiple of 128.

    The key-padding mask is known on the host, so masked keys are compacted
    away before the kernel runs: K/V/ebm arrive with only the kept keys
    (zero-padded to SP).  Padding rows have ebm == 0, so their probability
    is exactly 0 — identical math to masking, at ~half the work.
    """
    import concourse.bass as bass
    import concourse.mybir as mybir
    import concourse.tile as tile
    from concourse import bacc
    from contextlib import ExitStack

    DT = mybir.dt.bfloat16
    F32 = mybir.dt.float32
    Act = mybir.ActivationFunctionType

    EC = E // 128        # 8 contraction chunks for projections
    MC = JD // 128       # 2 partition-chunks of the per-core head dims
    SC = SP // 128       # kept-key chunks
    NT512 = T // 512     # 4
    NT128 = T // 128     # 16
    NE512 = E // 512     # 2

    nc = bacc.Bacc("TRN2", target_bir_lowering=False, debug=False)

    xqT = nc.dram_tensor("xqT", [E, T], DT, kind="ExternalInput")
    xkT = nc.dram_tensor("xkT", [E, SP], DT, kind="ExternalInput")
    xvT = nc.dram_tensor("xvT", [E, SP], DT, kind="ExternalInput")
    ebm = nc.dram_tensor("ebm", [SP, T], DT, kind="ExternalInput")
    wqT = nc.dram_tensor("wqT", [E, JD], DT, kind="ExternalInput")
    wkT = nc.dram_tensor("wkT", [E, JD], DT, kind="ExternalInput")
    wvT = nc.dram_tensor("wvT", [E, JD], DT, kind="ExternalInput")
    woT = nc.dram_tensor("woT", [JD, E], DT, kind="ExternalInput")
    bqv = nc.dram_tensor("bqv", [128, MC], F32, kind="ExternalInput")
    bkv = nc.dram_tensor("bkv", [128, MC], F32, kind="ExternalInput")
    out_p = nc.dram_tensor("out_p", [T, E], F32, kind="ExternalOutput")

    with tile.TileContext(nc) as tc, ExitStack() as ctx:
        const = ctx.enter_context(tc.tile_pool(name="const", bufs=1))
        persist = ctx.enter_context(tc.tile_pool(name="persist", bufs=1))

        # ---- constants / persistent tiles ----
        wo_sb = const.tile([128, MC, E], DT)
        bq_sb = const.tile([128, MC], F32)
        nc.sync.dma_start(out=bq_sb[:], in_=bqv[:, :])
        bk_sb = const.tile([128, MC], F32)
        nc.sync.dma_start(out=bk_sb[:], in_=bkv[:, :])

        qT_sb = persist.tile([128, MC, T], DT)
        kT_sb = persist.tile([128, MC, SP], DT)
        vone_sb = persist.tile([128, SC, HEADS, D + 1], DT)
        onorm_sb = persist.tile([128, MC, T], DT)
        ebm_sb = persist.tile([128, SC, T], DT)

        nc.vector.memset(vone_sb[:, :, :, D : D + 1], 1.0)

        # ---- phase 1: input projections ----
        with (
            nc.named_scope("proj"),
            tc.tile_pool(name="wpool", bufs=1) as wpool,
            tc.tile_pool(name="xpool", bufs=4) as xpool,
            tc.tile_pool(name="psA", bufs=4, space="PSUM") as psA,
        ):
            wq_sb = wpool.tile([128, EC, JD], DT)
            nc.sync.dma_start(
                out=wq_sb[:], in_=wqT.rearrange("(c p) j -> p c j", p=128)
            )
            wk_sb = wpool.tile([128, EC, JD], DT)
            nc.sync.dma_start(
                out=wk_sb[:], in_=wkT.rearrange("(c p) j -> p c j", p=128)
            )
            wv_sb = wpool.tile([128, EC, JD], DT)
            nc.sync.dma_start(
                out=wv_sb[:], in_=wvT.rearrange("(c p) j -> p c j", p=128)
            )
            HEC = EC // 2
            for w_sb, b_sb, xdram, dst, ncols in (
                (wq_sb, bq_sb, xqT, qT_sb, T),
                (wk_sb, bk_sb, xkT, kT_sb, SP),
            ):
                xr = xdram.rearrange("(c p) t -> c p t", p=128)
                xh = []
                for half in range(2):
                    t = xpool.tile(
                        [128, HEC, ncols], DT, tag="xhalf", name=f"xf_{dst.name}{half}"
                    )
                    for q4 in range(2):
                        nc.sync.dma_start(
                            out=t[:, q4 * (HEC // 2) : (q4 + 1) * (HEC // 2), :],
                            in_=xr[
                                half * HEC + q4 * (HEC // 2) : half * HEC
                                + (q4 + 1) * (HEC // 2)
                            ].rearrange("c p t -> p c t"),
                        )
                    xh.append(t)
                for mc in range(MC):
                    for t0 in range(0, ncols, 512):
                        w512 = min(512, ncols - t0)
                        ps = psA.tile([128, 512], F32, tag="pqk")
                        for ec in range(EC):
                            nc.tensor.matmul(
                                ps[:, 0:w512],
                                lhsT=w_sb[:, ec, mc * 128 : (mc + 1) * 128],
                                rhs=xh[ec // HEC][:, ec % HEC, t0 : t0 + w512],
                                start=(ec == 0),
                                stop=(ec == EC - 1),
                            )
                        nc.scalar.activation(
                            dst[:, mc, t0 : t0 + w512],
                            ps[:, 0:w512],
                            Act.Identity,
                            bias=b_sb[:, mc : mc + 1],
                        )

            xr = xvT.rearrange("(c p) t -> c p t", p=128)
            xh = []
            for half in range(2):
                t = xpool.tile([128, HEC, SP], DT, tag="xhalf", name=f"xf_v{half}")
                nc.sync.dma_start(
                    out=t[:],
                    in_=xr[half * HEC : (half + 1) * HEC].rearrange("c p t -> p c t"),
                )
                xh.append(t)
            for sc in range(SC):
                ps = psA.tile([128, JD], F32, tag="pv")
                for ec in range(EC):
                    nc.tensor.matmul(
                        ps[:],
                        lhsT=xh[ec // HEC][:, ec % HEC, sc * 128 : (sc + 1) * 128],
                        rhs=wv_sb[:, ec, :],
                        start=(ec == 0),
                        stop=(ec == EC - 1),
                    )
                nc.vector.tensor_copy(
                    vone_sb[:, sc, :, 0:D],
                    ps.rearrange("p (h d) -> p h d", h=HEADS),
                )

        # ebm load (phase 2 input); chunked so early chunks unblock head 0
        for sc in range(SC):
            nc.sync.dma_start(
                out=ebm_sb[:, sc, :],
                in_=ebm[sc * 128 : (sc + 1) * 128, :],
            )

        nc.sync.dma_start(out=wo_sb[:], in_=woT.rearrange("(c p) e -> p c e", p=128))

        # ---- phase 2+3: attention (head pairs) + fused out-projection ----
        with (
            nc.named_scope("attn"),
            tc.tile_pool(name="flow", bufs=8) as flow,
            tc.tile_pool(name="dpool", bufs=6) as dpool,
            tc.tile_pool(name="outp", bufs=4) as outp,
            tc.tile_pool(name="psB", bufs=2, space="PSUM") as psB,
            tc.tile_pool(name="psPV", bufs=4, space="PSUM") as psPV,
        ):
            # heads processed in pairs (2hp, 2hp+1): the two K=64 score
            # matmuls land on disjoint PE row-groups (partitions 0-63 /
            # 64-127) and run concurrently.  t is processed in halves of
            # 1024 so scores (4 banks) + the pair's PV accumulators
            # (4 banks) fit PSUM exactly.  After both pairs finish a
            # t-half, its out-projection runs (PSUM slots shared with the
            # scores tag), overlapping the next t-half's attention.
            for th in range(T // 1024):
                for hp in range(HEADS // 2):
                    mc = hp
                    pvp = {
                        (hh, j): psPV.tile(
                            [D + 1, 512], F32, tag="pvh", name=f"pvp{hp}_{th}_{hh}{j}"
                        )
                        for hh in range(2)
                        for j in range(2)
                    }
                    for sc in range(SC):
                        for j in range(2):
                            t0 = th * 1024 + j * 512
                            ps = psB.tile([128, 1024], F32, tag="sc")
                            for hh in range(2):
                                par = hh * 64
                                nc.tensor.matmul(
                                    ps[:, hh * 512 : (hh + 1) * 512],
                                    lhsT=kT_sb[
                                        par : par + 64, mc, sc * 128 : (sc + 1) * 128
                                    ],
                                    rhs=qT_sb[par : par + 64, mc, t0 : t0 + 512],
                                    start=True,
                                    stop=True,
                                )
                            pe = flow.tile([128, 1024], DT, tag="pe")
                            nc.scalar.activation(pe[:], ps[:], Act.Exp)
                            pw = flow.tile([128, 1024], DT, tag="pw")
                            nc.vector.tensor_mul(
                                pw.rearrange("p (c t) -> p c t", c=2),
                                pe.rearrange("p (c t) -> p c t", c=2),
                                ebm_sb[:, sc, t0 : t0 + 512].rearrange(
                                    "p (c t) -> p c t", c=1
                                ).broadcast_to([128, 2, 512]),
                            )
                            for hh in range(2):
                                nc.tensor.matmul(
                                    pvp[(hh, j)][:],
                                    lhsT=vone_sb[:, sc, 2 * hp + hh, :],
                                    rhs=pw[:, hh * 512 : (hh + 1) * 512],
                                    start=(sc == 0),
                                    stop=(sc == SC - 1),
                                )
                    for hh in range(2):
                        par = hh * 64
                        for j in range(2):
                            tq = th * 2 + j
                            pv = pvp[(hh, j)]
                            rec = dpool.tile([1, 512], F32, tag="rec")
                            nc.vector.reciprocal(rec[:], pv[D : D + 1, :])
                            bc = dpool.tile([64, 512], F32, tag="bc")
                            nc.gpsimd.partition_broadcast(bc[:], rec[:], channels=64)
                            nc.vector.tensor_mul(
                                onorm_sb[
                                    par : par + 64, mc, tq * 512 : (tq + 1) * 512
                                ],
                                pv[0:D, :],
                                bc[:],
                            )

                # out-projection for this t-half (all 4 heads drained);
                # uses the PV psum slots, which are free here, so it
                # overlaps the next t-half's scores/exp instead of
                # contending for their psum.
                for ti in range(th * 8, (th + 1) * 8):
                    ot = outp.tile([128, 1024], F32, tag="ot")
                    for ei in range(NE512):
                        po = psPV.tile(
                            [128, 512], F32, tag="pvh", name=f"po_{ti}_{ei}"
                        )
                        for mc2 in range(MC):
                            nc.tensor.matmul(
                                po[:],
                                lhsT=onorm_sb[:, mc2, ti * 128 : (ti + 1) * 128],
                                rhs=wo_sb[:, mc2, ei * 512 : (ei + 1) * 512],
                                start=(mc2 == 0),
                                stop=(mc2 == MC - 1),
                            )
                        nc.vector.tensor_copy(
                            ot[:, ei * 512 : (ei + 1) * 512], po[:]
                        )
                    nc.sync.dma_start(
                        out=out_p[ti * 128 : (ti + 1) * 128, :],
                        in_=ot[:],
                    )

    nc.compile()
    return nc


_NC_CACHE = {}
_NC = None
_LAST_RES = None


def _get_nc(SP=S):
    global _NC
    if SP not in _NC_CACHE:
        _NC_CACHE[SP] = _build_nc(SP)
    _NC = _NC_CACHE[SP]
    return _NC


def kernel(query, key, value, attn_bias, key_padding_mask,
           in_proj_w, in_proj_b, out_proj_w, out_proj_b):
    from concourse.bass_utils import run_bass_kernel_spmd

    query = np.asarray(query, np.float32)
    key = np.asarray(key, np.float32)
    value = np.asarray(value, np.float32)
    attn_bias = np.asarray(attn_bias, np.float32)
    key_padding_mask = np.asarray(key_padding_mask, bool)
    in_proj_w = np.asarray(in_proj_w, np.float32)
    in_proj_b = np.asarray(in_proj_b, np.float32)
    out_proj_w = np.asarray(out_proj_w, np.float32)
    out_proj_b = np.asarray(out_proj_b, np.float32)

    w_q, w_k, w_v = in_proj_w[:E], in_proj_w[E : 2 * E], in_proj_w[2 * E :]
    b_q, b_k, b_v = in_proj_b[:E], in_proj_b[E : 2 * E], in_proj_b[2 * E :]

    ebm_base = np.exp(attn_bias[0]).T  # [S, T]

    # compact away masked keys (their softmax weight is exactly 0);
    # pad the kept set to a multiple of 128 with ebm == 0 rows.
    idx = {b: np.nonzero(~key_padding_mask[b])[0] for b in range(B)}
    s_eff = max(len(idx[b]) for b in range(B))
    SP = max(128, -(-s_eff // 128) * 128)

    xT = {}
    for b in range(B):
        n = len(idx[b])
        kc = np.zeros((SP, E), np.float32)
        kc[:n] = key[b][idx[b]]
        vc = np.zeros((SP, E), np.float32)
        vc[:n] = value[b][idx[b]]
        ec = np.zeros((SP, T), np.float32)
        ec[:n] = ebm_base[idx[b]]
        xT[b] = (
            query[b].T.astype(BF16),
            kc.T.astype(BF16),
            vc.T.astype(BF16),
            ec.astype(BF16),
        )

    in_maps = []
    for core in range(NCORES):
        b, hg = core // 4, core % 4
        rows = slice(hg * JD, (hg + 1) * JD)
        qT, kT, vT, ebm_b = xT[b]
        in_maps.append({
            "xqT": qT, "xkT": kT, "xvT": vT, "ebm": ebm_b,
            "wqT": np.ascontiguousarray((SCALING * w_q[rows]).T).astype(BF16),
            "wkT": np.ascontiguousarray(w_k[rows].T).astype(BF16),
            "wvT": np.ascontiguousarray(w_v[rows].T).astype(BF16),
            "woT": np.ascontiguousarray(out_proj_w[:, rows].T).astype(BF16),
            "bqv": np.ascontiguousarray(
                (SCALING * b_q[rows]).reshape(JD // 128, 128).T),
            "bkv": np.ascontiguousarray(b_k[rows].reshape(JD // 128, 128).T),
        })

    nc = _get_nc(SP)
    import os
    trace = os.environ.get("KERNEL_TRACE", "") == "1"
    kwargs = {}
    if trace:
        kwargs["tmpdir"] = os.environ.get("KERNEL_TRACE_DIR") or None
    res = run_bass_kernel_spmd(
        nc, in_maps, core_ids=list(range(NCORES)), trace=trace, **kwargs
    )
    global _LAST_RES
    _LAST_RES = res

    out = np.zeros((B, T, E), np.float32)
    for core in range(NCORES):
        out[core // 4] += res.results[core]["out_p"]
    out += (out_proj_b + out_proj_w @ b_v)[None, None, :]
    return out


# revision 34
# speedup vs baseline: 61772.3472x; 1.0591x over previous
"""Multi-head attention (B=2, T=S=2048, E=1024, H=16, D=64) on 8 NeuronCores.

Sharding: core = (batch, head-group-of-4).  Each core computes the full
attention for 4 heads of one batch plus that slice's out-projection
contribution; the host sums the 4 partials per batch.

Device math (per core, all matmuls bf16, accumulation fp32 in PSUM):
  qT = (scale*Wq_c) @ x_q     -> [256, T]   (d on partitions)
  kT = Wk_c @ x_k             -> [256, S]
  v  = x_v @ Wv_c^T           -> [S, 256]   (s on partitions), plus ones col
  scoresT[s,t] = kT^T·qT      (K=64 per head)
  p = exp(scoresT) * ebm      (ebm = exp(bias^T)*keep_mask, host-precomputed;
                               softmax needs no max-subtraction: scores are O(10))
  [out_unT; sums] = [v|1]^T @ p   (ones column fuses the softmax denominator)
  onorm = out_unT * bcast(1/sums) (gpsimd partition-broadcast)
  out_partial = onorm^T @ Wo_c^T  -> [T, E] fp32
"""

import numpy as np
import ml_dtypes

B, T, S, E = 2, 2048, 2048, 1024
H, D = 16, 64
SCALING = float(D) ** -0.5
HEADS = 4              # heads per core
JD = HEADS * D         # 256 projected dims per core
NCORES = 8

BF16 = ml_dtypes.bfloat16


def _build_nc(SP=S):
    """SP = padded count of unmasked (kept) keys, multiple of 128.

    The key-padding mask is known on the host, so masked keys are compacted
    away before the kernel runs: K/V/ebm arrive with only the kept keys
    (zero-padded to SP).  Padding rows have ebm == 0, so their probability
    is exactly 0 — identical math to masking, at ~half the work.
    """
    import concourse.bass as bass
    import concourse.mybir as mybir
    import concourse.tile as tile
    from concourse import bacc
    from contextlib import ExitStack

    DT = mybir.dt.bfloat16
    F32 = mybir.dt.float32
    Act = mybir.ActivationFunctionType

    EC = E // 128        # 8 contraction chunks for projections
    MC = JD // 128       # 2 partition-chunks of the per-core head dims
    SC = SP // 128       # kept-key chunks
    NT512 = T // 512     # 4
    NT128 = T // 128     # 16
    NE512 = E // 512     # 2

    nc = bacc.Bacc("TRN2", target_bir_lowering=False, debug=False)

    xqT = nc.dram_tensor("xqT", [E, T], DT, kind="ExternalInput")
    xkT = nc.dram_tensor("xkT", [E, SP], DT, kind="ExternalInput")
    xvT = nc.dram_tensor("xvT", [E, SP], DT, kind="ExternalInput")
    ebm = nc.dram_tensor("ebm", [SP, T], DT, kind="ExternalInput")
    wqT = nc.dram_tensor("wqT", [E, JD], DT, kind="ExternalInput")
    wkT = nc.dram_tensor("wkT", [E, JD], DT, kind="ExternalInput")
    wvT = nc.dram_tensor("wvT", [E, JD], DT, kind="ExternalInput")
    woT = nc.dram_tensor("woT", [JD, E], DT, kind="ExternalInput")
    bqv = nc.dram_tensor("bqv", [128, MC], F32, kind="ExternalInput")
    bkv = nc.dram_tensor("bkv", [128, MC], F32, kind="ExternalInput")
    out_p = nc.dram_tensor("out_p", [T, E], F32, kind="ExternalOutput")

    with tile.TileContext(nc) as tc, ExitStack() as ctx:
        const = ctx.enter_context(tc.tile_pool(name="const", bufs=1))
        persist = ctx.enter_context(tc.tile_pool(name="persist", bufs=1))

        # ---- constants / persistent tiles ----
        wo_sb = const.tile([128, MC, E], DT)
        bq_sb = const.tile([128, MC], F32)
        nc.sync.dma_start(out=bq_sb[:], in_=bqv[:, :])
        bk_sb = const.tile([128, MC], F32)
        nc.sync.dma_start(out=bk_sb[:], in_=bkv[:, :])

        qT_sb = persist.tile([128, MC, T], DT)
        kT_sb = persist.tile([128, MC, SP], DT)
        vone_sb = persist.tile([128, SC, HEADS, D + 1], DT)
        onorm_sb = persist.tile([128, MC, T], DT)
        ebm_sb = persist.tile([128, SC, T], DT)

        nc.vector.memset(vone_sb[:, :, :, D : D + 1], 1.0)

        # ---- phase 1: input projections ----
        with (
            nc.named_scope("proj"),
            tc.tile_pool(name="wpool", bufs=1) as wpool,
            tc.tile_pool(name="xpool", bufs=4) as xpool,
            tc.tile_pool(name="psA", bufs=4, space="PSUM") as psA,
        ):
            wq_sb = wpool.tile([128, EC, JD], DT)
            nc.sync.dma_start(
                out=wq_sb[:], in_=wqT.rearrange("(c p) j -> p c j", p=128)
            )
            wk_sb = wpool.tile([128, EC, JD], DT)
            wv_sb = wpool.tile([128, EC, JD], DT)
            HEC = EC // 2
            for w_sb, b_sb, xdram, dst, ncols in (
                (wq_sb, bq_sb, xqT, qT_sb, T),
                (wk_sb, bk_sb, xkT, kT_sb, SP),
            ):
                if w_sb is wk_sb:
                    # deferred so the startup DMA budget goes to xq first
                    nc.sync.dma_start(
                        out=wk_sb[:], in_=wkT.rearrange("(c p) j -> p c j", p=128)
                    )
                xr = xdram.rearrange("(c p) t -> c p t", p=128)
                xh = []
                for half in range(2):
                    t = xpool.tile(
                        [128, HEC, ncols], DT, tag="xhalf", name=f"xf_{dst.name}{half}"
                    )
                    for q4 in range(2):
                        nc.sync.dma_start(
                            out=t[:, q4 * (HEC // 2) : (q4 + 1) * (HEC // 2), :],
                            in_=xr[
                                half * HEC + q4 * (HEC // 2) : half * HEC
                                + (q4 + 1) * (HEC // 2)
                            ].rearrange("c p t -> p c t"),
                        )
                    xh.append(t)
                for mc in range(MC):
                    for t0 in range(0, ncols, 512):
                        w512 = min(512, ncols - t0)
                        ps = psA.tile([128, 512], F32, tag="pqk")
                        for ec in range(EC):
                            nc.tensor.matmul(
                                ps[:, 0:w512],
                                lhsT=w_sb[:, ec, mc * 128 : (mc + 1) * 128],
                                rhs=xh[ec // HEC][:, ec % HEC, t0 : t0 + w512],
                                start=(ec == 0),
                                stop=(ec == EC - 1),
                            )
                        nc.scalar.activation(
                            dst[:, mc, t0 : t0 + w512],
                            ps[:, 0:w512],
                            Act.Identity,
                            bias=b_sb[:, mc : mc + 1],
                        )

            nc.sync.dma_start(
                out=wv_sb[:], in_=wvT.rearrange("(c p) j -> p c j", p=128)
            )
            xr = xvT.rearrange("(c p) t -> c p t", p=128)
            xh = []
            for half in range(2):
                t = xpool.tile([128, HEC, SP], DT, tag="xhalf", name=f"xf_v{half}")
                nc.sync.dma_start(
                    out=t[:],
                    in_=xr[half * HEC : (half + 1) * HEC].rearrange("c p t -> p c t"),
                )
                xh.append(t)
            for sc in range(SC):
                ps = psA.tile([128, JD], F32, tag="pv")
                for ec in range(EC):
                    nc.tensor.matmul(
                        ps[:],
                        lhsT=xh[ec // HEC][:, ec % HEC, sc * 128 : (sc + 1) * 128],
                        rhs=wv_sb[:, ec, :],
                        start=(ec == 0),
                        stop=(ec == EC - 1),
                    )
                nc.vector.tensor_copy(
                    vone_sb[:, sc, :, 0:D],
                    ps.rearrange("p (h d) -> p h d", h=HEADS),
                )

        # ebm load (phase 2 input); chunked so early chunks unblock head 0
        for sc in range(SC):
            nc.sync.dma_start(
                out=ebm_sb[:, sc, :],
                in_=ebm[sc * 128 : (sc + 1) * 128, :],
            )

        nc.sync.dma_start(out=wo_sb[:], in_=woT.rearrange("(c p) e -> p c e", p=128))

        # ---- phase 2+3: attention (head pairs) + fused out-projection ----
        with (
            nc.named_scope("attn"),
            tc.tile_pool(name="flow", bufs=8) as flow,
            tc.tile_pool(name="dpool", bufs=6) as dpool,
            tc.tile_pool(name="outp", bufs=4) as outp,
            tc.tile_pool(name="psB", bufs=2, space="PSUM") as psB,
            tc.tile_pool(name="psPV", bufs=4, space="PSUM") as psPV,
        ):
            # heads processed in pairs (2hp, 2hp+1): the two K=64 score
            # matmuls land on disjoint PE row-groups (partitions 0-63 /
            # 64-127) and run concurrently.  t is processed in halves of
            # 1024 so scores (4 banks) + the pair's PV accumulators
            # (4 banks) fit PSUM exactly.  After both pairs finish a
            # t-half, its out-projection runs (PSUM slots shared with the
            # scores tag), overlapping the next t-half's attention.
            for th in range(T // 1024):
                for hp in range(HEADS // 2):
                    mc = hp
                    pvp = {
                        (hh, j): psPV.tile(
                            [D + 1, 512], F32, tag="pvh", name=f"pvp{hp}_{th}_{hh}{j}"
                        )
                        for hh in range(2)
                        for j in range(2)
                    }
                    for sc in range(SC):
                        for j in range(2):
                            t0 = th * 1024 + j * 512
                            ps = psB.tile([128, 1024], F32, tag="sc")
                            for hh in range(2):
                                par = hh * 64
                                nc.tensor.matmul(
                                    ps[:, hh * 512 : (hh + 1) * 512],
                                    lhsT=kT_sb[
                                        par : par + 64, mc, sc * 128 : (sc + 1) * 128
                                    ],
                                    rhs=qT_sb[par : par + 64, mc, t0 : t0 + 512],
                                    start=True,
                                    stop=True,
                                )
                            pe = flow.tile([128, 1024], DT, tag="pe")
                            nc.scalar.activation(pe[:], ps[:], Act.Exp)
                            pw = flow.tile([128, 1024], DT, tag="pw")
                            nc.vector.tensor_mul(
                                pw.rearrange("p (c t) -> p c t", c=2),
                                pe.rearrange("p (c t) -> p c t", c=2),
                                ebm_sb[:, sc, t0 : t0 + 512].rearrange(
                                    "p (c t) -> p c t", c=1
                                ).broadcast_to([128, 2, 512]),
                            )
                            for hh in range(2):
                                nc.tensor.matmul(
                                    pvp[(hh, j)][:],
                                    lhsT=vone_sb[:, sc, 2 * hp + hh, :],
                                    rhs=pw[:, hh * 512 : (hh + 1) * 512],
                                    start=(sc == 0),
                                    stop=(sc == SC - 1),
                                )
                    for hh in range(2):
                        par = hh * 64
                        for j in range(2):
                            tq = th * 2 + j
                            pv = pvp[(hh, j)]
                            rec = dpool.tile([1, 512], F32, tag="rec")
                            nc.vector.reciprocal(rec[:], pv[D : D + 1, :])
                            bc = dpool.tile([64, 512], F32, tag="bc")
                            nc.gpsimd.partition_broadcast(bc[:], rec[:], channels=64)
                            nc.vector.tensor_mul(
                                onorm_sb[
                                    par : par + 64, mc, tq * 512 : (tq + 1) * 512
                                ],
                                pv[0:D, :],
                                bc[:],
                            )

                # out-projection for this t-half (all 4 heads drained);
                # uses the PV psum slots, which are free here, so it
                # overlaps the next t-half's scores/exp instead of
                # contending for their psum.
                for ti in range(th * 8, (th + 1) * 8):
                    ot = outp.tile([128, 1024], F32, tag="ot")
                    for ei in range(NE512):
                        po = psPV.tile(
                            [128, 512], F32, tag="pvh", name=f"po_{ti}_{ei}"
                        )
                        for mc2 in range(MC):
                            nc.tensor.matmul(
                                po[:],
                                lhsT=onorm_sb[:, mc2, ti * 128 : (ti + 1) * 128],
                                rhs=wo_sb[:, mc2, ei * 512 : (ei + 1) * 512],
                                start=(mc2 == 0),
                                stop=(mc2 == MC - 1),
                            )
                        nc.scalar.copy(ot[:, ei * 512 : (ei + 1) * 512], po[:])
                    nc.sync.dma_start(
                        out=out_p[ti * 128 : (ti + 1) * 128, :],
                        in_=ot[:],
                    )

    nc.compile()
    return nc


_NC_CACHE = {}
_NC = None
_LAST_RES = None


def _get_nc(SP=S):
    global _NC
    if SP not in _NC_CACHE:
        _NC_CACHE[SP] = _build_nc(SP)
    _NC = _NC_CACHE[SP]
    return _NC


def kernel(query, key, value, attn_bias, key_padding_mask,
           in_proj_w, in_proj_b, out_proj_w, out_proj_b):
    from concourse.bass_utils import run_bass_kernel_spmd

    query = np.asarray(query, np.float32)
    key = np.asarray(key, np.float32)
    value = np.asarray(value, np.float32)
    attn_bias = np.asarray(attn_bias, np.float32)
    key_padding_mask = np.asarray(key_padding_mask, bool)
    in_proj_w = np.asarray(in_proj_w, np.float32)
    in_proj_b = np.asarray(in_proj_b, np.float32)
    out_proj_w = np.asarray(out_proj_w, np.float32)
    out_proj_b = np.asarray(out_proj_b, np.float32)

    w_q, w_k, w_v = in_proj_w[:E], in_proj_w[E : 2 * E], in_proj_w[2 * E :]
    b_q, b_k, b_v = in_proj_b[:E], in_proj_b[E : 2 * E], in_proj_b[2 * E :]

    ebm_base = np.exp(attn_bias[0]).T  # [S, T]

    # compact away masked keys (their softmax weight is exactly 0);
    # pad the kept set to a multiple of 128 with ebm == 0 rows.
    idx = {b: np.nonzero(~key_padding_mask[b])[0] for b in range(B)}
    s_eff = max(len(idx[b]) for b in range(B))
    SP = max(128, -(-s_eff // 128) * 128)

    xT = {}
    for b in range(B):
        n = len(idx[b])
        kc = np.zeros((SP, E), np.float32)
        kc[:n] = key[b][idx[b]]
        vc = np.zeros((SP, E), np.float32)
        vc[:n] = value[b][idx[b]]
        ec = np.zeros((SP, T), np.float32)
        ec[:n] = ebm_base[idx[b]]
        xT[b] = (
            query[b].T.astype(BF16),
            kc.T.astype(BF16),
            vc.T.astype(BF16),
            ec.astype(BF16),
        )

    in_maps = []
    for core in range(NCORES):
        b, hg = core // 4, core % 4
        rows = slice(hg * JD, (hg + 1) * JD)
        qT, kT, vT, ebm_b = xT[b]
        in_maps.append({
            "xqT": qT, "xkT": kT, "xvT": vT, "ebm": ebm_b,
            "wqT": np.ascontiguousarray((SCALING * w_q[rows]).T).astype(BF16),
            "wkT": np.ascontiguousarray(w_k[rows].T).astype(BF16),
            "wvT": np.ascontiguousarray(w_v[rows].T).astype(BF16),
            "woT": np.ascontiguousarray(out_proj_w[:, rows].T).astype(BF16),
            "bqv": np.ascontiguousarray(
                (SCALING * b_q[rows]).reshape(JD // 128, 128).T),
            "bkv": np.ascontiguousarray(b_k[rows].reshape(JD // 128, 128).T),
        })

    nc = _get_nc(SP)
    import os
    trace = os.environ.get("KERNEL_TRACE", "") == "1"
    kwargs = {}
    if trace:
        kwargs["tmpdir"] = os.environ.get("KERNEL_TRACE_DIR") or None
    res = run_bass_kernel_spmd(
        nc, in_maps, core_ids=list(range(NCORES)), trace=trace, **kwargs
    )
    global _LAST_RES
    _LAST_RES = res

    out = np.zeros((B, T, E), np.float32)
    for core in range(NCORES):
        out[core // 4] += res.results[core]["out_p"]
    out += (out_proj_b + out_proj_w @ b_v)[None, None, :]
    return out


# revision 36
# speedup vs baseline: 62074.9382x; 1.0049x over previous
"""Multi-head attention (B=2, T=S=2048, E=1024, H=16, D=64) on 8 NeuronCores.

Sharding: core = (batch, head-group-of-4).  Each core computes the full
attention for 4 heads of one batch plus that slice's out-projection
contribution; the host sums the 4 partials per batch.

Device math (per core, all matmuls bf16, accumulation fp32 in PSUM):
  qT = (scale*Wq_c) @ x_q     -> [256, T]   (d on partitions)
  kT = Wk_c @ x_k             -> [256, S]
  v  = x_v @ Wv_c^T           -> [S, 256]   (s on partitions), plus ones col
  scoresT[s,t] = kT^T·qT      (K=64 per head)
  p = exp(scoresT) * ebm      (ebm = exp(bias^T)*keep_mask, host-precomputed;
                               softmax needs no max-subtraction: scores are O(10))
  [out_unT; sums] = [v|1]^T @ p   (ones column fuses the softmax denominator)
  onorm = out_unT * bcast(1/sums) (gpsimd partition-broadcast)
  out_partial = onorm^T @ Wo_c^T  -> [T, E] fp32
"""

import numpy as np
import ml_dtypes

B, T, S, E = 2, 2048, 2048, 1024
H, D = 16, 64
SCALING = float(D) ** -0.5
HEADS = 4              # heads per core
JD = HEADS * D         # 256 projected dims per core
NCORES = 8

BF16 = ml_dtypes.bfloat16


def _build_nc(SP=S):
    """SP = padded count of unmasked (kept) keys, multiple of 128.

    The key-padding mask is known on the host, so masked keys are compacted
    away before the kernel runs: K/V/ebm arrive with only the kept keys
    (zero-padded to SP).  Padding rows have ebm == 0, so their probability
    is exactly 0 — identical math to masking, at ~half the work.
    """
    import concourse.bass as bass
    import concourse.mybir as mybir
    import concourse.tile as tile
    from concourse import bacc
    from contextlib import ExitStack

    DT = mybir.dt.bfloat16
    F32 = mybir.dt.float32
    Act = mybir.ActivationFunctionType

    EC = E // 128        # 8 contraction chunks for projections
    MC = JD // 128       # 2 partition-chunks of the per-core head dims
    SC = SP // 128       # kept-key chunks
    NT512 = T // 512     # 4
    NT128 = T // 128     # 16
    NE512 = E // 512     # 2

    nc = bacc.Bacc("TRN2", target_bir_lowering=False, debug=False)

    xqT = nc.dram_tensor("xqT", [E, T], DT, kind="ExternalInput")
    xkT = nc.dram_tensor("xkT", [E, SP], DT, kind="ExternalInput")
    xvT = nc.dram_tensor("xvT", [E, SP], DT, kind="ExternalInput")
    ebm = nc.dram_tensor("ebm", [SP, T], DT, kind="ExternalInput")
    wqT = nc.dram_tensor("wqT", [E, JD], DT, kind="ExternalInput")
    wkT = nc.dram_tensor("wkT", [E, JD], DT, kind="ExternalInput")
    wvT = nc.dram_tensor("wvT", [E, JD], DT, kind="ExternalInput")
    woT = nc.dram_tensor("woT", [JD, E], DT, kind="ExternalInput")
    bqv = nc.dram_tensor("bqv", [128, MC], F32, kind="ExternalInput")
    bkv = nc.dram_tensor("bkv", [128, MC], F32, kind="ExternalInput")
    out_p = nc.dram_tensor("out_p", [T, E], F32, kind="ExternalOutput")

    with tile.TileContext(nc) as tc, ExitStack() as ctx:
        const = ctx.enter_context(tc.tile_pool(name="const", bufs=1))
        persist = ctx.enter_context(tc.tile_pool(name="persist", bufs=1))

        # ---- constants / persistent tiles ----
        wo_sb = const.tile([128, MC, E], DT)
        bq_sb = const.tile([128, MC], F32)
        nc.sync.dma_start(out=bq_sb[:], in_=bqv[:, :])
        bk_sb = const.tile([128, MC], F32)
        nc.sync.dma_start(out=bk_sb[:], in_=bkv[:, :])

        qT_sb = persist.tile([128, MC, T], DT)
        kT_sb = persist.tile([128, MC, SP], DT)
        vone_sb = persist.tile([128, SC, HEADS, D + 1], DT)
        onorm_sb = persist.tile([128, MC, T], DT)
        ebm_sb = persist.tile([128, SC, T], DT)

        nc.vector.memset(vone_sb[:, :, :, D : D + 1], 1.0)

        # ---- phase 1: input projections ----
        with (
            nc.named_scope("proj"),
            tc.tile_pool(name="wpool", bufs=1) as wpool,
            tc.tile_pool(name="xpool", bufs=4) as xpool,
            tc.tile_pool(name="psA", bufs=4, space="PSUM") as psA,
        ):
            wq_sb = wpool.tile([128, EC, JD], DT)
            wk_sb = wpool.tile([128, EC, JD], DT)
            wv_sb = wpool.tile([128, EC, JD], DT)
            # K first: its x-load is the smallest (SP cols), so PE starts
            # soonest; Q (T cols, largest load) overlaps K's compute.
            nc.sync.dma_start(
                out=wk_sb[:], in_=wkT.rearrange("(c p) j -> p c j", p=128)
            )
            HEC = EC // 2
            for w_sb, b_sb, xdram, dst, ncols in (
                (wk_sb, bk_sb, xkT, kT_sb, SP),
                (wq_sb, bq_sb, xqT, qT_sb, T),
            ):
                if w_sb is wq_sb:
                    nc.sync.dma_start(
                        out=wq_sb[:], in_=wqT.rearrange("(c p) j -> p c j", p=128)
                    )
                xr = xdram.rearrange("(c p) t -> c p t", p=128)
                xh = []
                for half in range(2):
                    t = xpool.tile(
                        [128, HEC, ncols], DT, tag="xhalf", name=f"xf_{dst.name}{half}"
                    )
                    for q4 in range(2):
                        nc.sync.dma_start(
                            out=t[:, q4 * (HEC // 2) : (q4 + 1) * (HEC // 2), :],
                            in_=xr[
                                half * HEC + q4 * (HEC // 2) : half * HEC
                                + (q4 + 1) * (HEC // 2)
                            ].rearrange("c p t -> p c t"),
                        )
                    xh.append(t)
                for mc in range(MC):
                    for t0 in range(0, ncols, 512):
                        w512 = min(512, ncols - t0)
                        ps = psA.tile([128, 512], F32, tag="pqk")
                        for ec in range(EC):
                            nc.tensor.matmul(
                                ps[:, 0:w512],
                                lhsT=w_sb[:, ec, mc * 128 : (mc + 1) * 128],
                                rhs=xh[ec // HEC][:, ec % HEC, t0 : t0 + w512],
                                start=(ec == 0),
                                stop=(ec == EC - 1),
                            )
                        nc.scalar.activation(
                            dst[:, mc, t0 : t0 + w512],
                            ps[:, 0:w512],
                            Act.Identity,
                            bias=b_sb[:, mc : mc + 1],
                        )

            nc.sync.dma_start(
                out=wv_sb[:], in_=wvT.rearrange("(c p) j -> p c j", p=128)
            )
            xr = xvT.rearrange("(c p) t -> c p t", p=128)
            xh = []
            for half in range(2):
                t = xpool.tile([128, HEC, SP], DT, tag="xhalf", name=f"xf_v{half}")
                nc.sync.dma_start(
                    out=t[:],
                    in_=xr[half * HEC : (half + 1) * HEC].rearrange("c p t -> p c t"),
                )
                xh.append(t)
            for sc in range(SC):
                ps = psA.tile([128, JD], F32, tag="pv")
                for ec in range(EC):
                    nc.tensor.matmul(
                        ps[:],
                        lhsT=xh[ec // HEC][:, ec % HEC, sc * 128 : (sc + 1) * 128],
                        rhs=wv_sb[:, ec, :],
                        start=(ec == 0),
                        stop=(ec == EC - 1),
                    )
                nc.vector.tensor_copy(
                    vone_sb[:, sc, :, 0:D],
                    ps.rearrange("p (h d) -> p h d", h=HEADS),
                )

        # ebm load (phase 2 input); chunked so early chunks unblock head 0
        for sc in range(SC):
            nc.sync.dma_start(
                out=ebm_sb[:, sc, :],
                in_=ebm[sc * 128 : (sc + 1) * 128, :],
            )

        nc.sync.dma_start(out=wo_sb[:], in_=woT.rearrange("(c p) e -> p c e", p=128))

        # ---- phase 2+3: attention (head pairs) + fused out-projection ----
        with (
            nc.named_scope("attn"),
            tc.tile_pool(name="flow", bufs=8) as flow,
            tc.tile_pool(name="dpool", bufs=6) as dpool,
            tc.tile_pool(name="outp", bufs=4) as outp,
            tc.tile_pool(name="psB", bufs=2, space="PSUM") as psB,
            tc.tile_pool(name="psPV", bufs=4, space="PSUM") as psPV,
        ):
            # heads processed in pairs (2hp, 2hp+1): the two K=64 score
            # matmuls land on disjoint PE row-groups (partitions 0-63 /
            # 64-127) and run concurrently.  t is processed in halves of
            # 1024 so scores (4 banks) + the pair's PV accumulators
            # (4 banks) fit PSUM exactly.  After both pairs finish a
            # t-half, its out-projection runs (PSUM slots shared with the
            # scores tag), overlapping the next t-half's attention.
            for th in range(T // 1024):
                for hp in range(HEADS // 2):
                    mc = hp
                    pvp = {
                        (hh, j): psPV.tile(
                            [D + 1, 512], F32, tag="pvh", name=f"pvp{hp}_{th}_{hh}{j}"
                        )
                        for hh in range(2)
                        for j in range(2)
                    }
                    for sc in range(SC):
                        for j in range(2):
                            t0 = th * 1024 + j * 512
                            ps = psB.tile([128, 1024], F32, tag="sc")
                            for hh in range(2):
                                par = hh * 64
                                nc.tensor.matmul(
                                    ps[:, hh * 512 : (hh + 1) * 512],
                                    lhsT=kT_sb[
                                        par : par + 64, mc, sc * 128 : (sc + 1) * 128
                                    ],
                                    rhs=qT_sb[par : par + 64, mc, t0 : t0 + 512],
                                    start=True,
                                    stop=True,
                                )
                            pe = flow.tile([128, 1024], DT, tag="pe")
                            nc.scalar.activation(pe[:], ps[:], Act.Exp)
                            pw = flow.tile([128, 1024], DT, tag="pw")
                            nc.vector.tensor_mul(
                                pw.rearrange("p (c t) -> p c t", c=2),
                                pe.rearrange("p (c t) -> p c t", c=2),
                                ebm_sb[:, sc, t0 : t0 + 512].rearrange(
                                    "p (c t) -> p c t", c=1
                                ).broadcast_to([128, 2, 512]),
                            )
                            for hh in range(2):
                                nc.tensor.matmul(
                                    pvp[(hh, j)][:],
                                    lhsT=vone_sb[:, sc, 2 * hp + hh, :],
                                    rhs=pw[:, hh * 512 : (hh + 1) * 512],
                                    start=(sc == 0),
                                    stop=(sc == SC - 1),
                                )
                    for hh in range(2):
                        par = hh * 64
                        for j in range(2):
                            tq = th * 2 + j
                            pv = pvp[(hh, j)]
                            rec = dpool.tile([1, 512], F32, tag="rec")
                            nc.vector.reciprocal(rec[:], pv[D : D + 1, :])
                            bc = dpool.tile([64, 512], F32, tag="bc")
                            nc.gpsimd.partition_broadcast(bc[:], rec[:], channels=64)
                            nc.vector.tensor_mul(
                                onorm_sb[
                                    par : par + 64, mc, tq * 512 : (tq + 1) * 512
                                ],
                                pv[0:D, :],
                                bc[:],
                            )

                # out-projection for this t-half (all 4 heads drained);
                # uses the PV psum slots, which are free here, so it
                # overlaps the next t-half's scores/exp instead of
                # contending for their psum.
                for ti in range(th * 8, (th + 1) * 8):
                    ot = outp.tile([128, 1024], F32, tag="ot")
                    for ei in range(NE512):
                        po = psPV.tile(
                            [128, 512], F32, tag="pvh", name=f"po_{ti}_{ei}"
                        )
                        for mc2 in range(MC):
                            nc.tensor.matmul(
                                po[:],
                                lhsT=onorm_sb[:, mc2, ti * 128 : (ti + 1) * 128],
                                rhs=wo_sb[:, mc2, ei * 512 : (ei + 1) * 512],
                                start=(mc2 == 0),
                                stop=(mc2 == MC - 1),
                            )
                        if ei == 0:
                            nc.scalar.copy(ot[:, 0:512], po[:])
                        else:
                            nc.vector.tensor_copy(
                                ot[:, ei * 512 : (ei + 1) * 512], po[:]
                            )
                    nc.sync.dma_start(
                        out=out_p[ti * 128 : (ti + 1) * 128, :],
                        in_=ot[:],
                    )

    nc.compile()
    return nc


_NC_CACHE = {}
_NC = None
_LAST_RES = None


def _get_nc(SP=S):
    global _NC
    if SP not in _NC_CACHE:
        _NC_CACHE[SP] = _build_nc(SP)
    _NC = _NC_CACHE[SP]
    return _NC


def kernel(query, key, value, attn_bias, key_padding_mask,
           in_proj_w, in_proj_b, out_proj_w, out_proj_b):
    from concourse.bass_utils import run_bass_kernel_spmd

    query = np.asarray(query, np.float32)
    key = np.asarray(key, np.float32)
    value = np.asarray(value, np.float32)
    attn_bias = np.asarray(attn_bias, np.float32)
    key_padding_mask = np.asarray(key_padding_mask, bool)
    in_proj_w = np.asarray(in_proj_w, np.float32)
    in_proj_b = np.asarray(in_proj_b, np.float32)
    out_proj_w = np.asarray(out_proj_w, np.float32)
    out_proj_b = np.asarray(out_proj_b, np.float32)

    w_q, w_k, w_v = in_proj_w[:E], in_proj_w[E : 2 * E], in_proj_w[2 * E :]
    b_q, b_k, b_v = in_proj_b[:E], in_proj_b[E : 2 * E], in_proj_b[2 * E :]

    ebm_base = np.exp(attn_bias[0]).T  # [S, T]

    # compact away masked keys (their softmax weight is exactly 0);
    # pad the kept set to a multiple of 128 with ebm == 0 rows.
    idx = {b: np.nonzero(~key_padding_mask[b])[0] for b in range(B)}
    s_eff = max(len(idx[b]) for b in range(B))
    SP = max(128, -(-s_eff // 128) * 128)

    xT = {}
    for b in range(B):
        n = len(idx[b])
        kc = np.zeros((SP, E), np.float32)
        kc[:n] = key[b][idx[b]]
        vc = np.zeros((SP, E), np.float32)
        vc[:n] = value[b][idx[b]]
        ec = np.zeros((SP, T), np.float32)
        ec[:n] = ebm_base[idx[b]]
        xT[b] = (
            query[b].T.astype(BF16),
            kc.T.astype(BF16),
            vc.T.astype(BF16),
            ec.astype(BF16),
        )

    in_maps = []
    for core in range(NCORES):
        b, hg = core // 4, core % 4
        rows = slice(hg * JD, (hg + 1) * JD)
        qT, kT, vT, ebm_b = xT[b]
        in_maps.append({
            "xqT": qT, "xkT": kT, "xvT": vT, "ebm": ebm_b,
            "wqT": np.ascontiguousarray((SCALING * w_q[rows]).T).astype(BF16),
            "wkT": np.ascontiguousarray(w_k[rows].T).astype(BF16),
            "wvT": np.ascontiguousarray(w_v[rows].T).astype(BF16),
            "woT": np.ascontiguousarray(out_proj_w[:, rows].T).astype(BF16),
            "bqv": np.ascontiguousarray(
                (SCALING * b_q[rows]).reshape(JD // 128, 128).T),
            "bkv": np.ascontiguousarray(b_k[rows].reshape(JD // 128, 128).T),
        })

    nc = _get_nc(SP)
    import os
    trace = os.environ.get("KERNEL_TRACE", "") == "1"
    kwargs = {}
    if trace:
        kwargs["tmpdir"] = os.environ.get("KERNEL_TRACE_DIR") or None
    res = run_bass_kernel_spmd(
        nc, in_maps, core_ids=list(range(NCORES)), trace=trace, **kwargs
    )
    global _LAST_RES
    _LAST_RES = res

    out = np.zeros((B, T, E), np.float32)
    for core in range(NCORES):
        out[core // 4] += res.results[core]["out_p"]
    out += (out_proj_b + out_proj_w @ b_v)[None, None, :]
    return out
